# revision 29
# baseline (speedup 1.0000x reference)
"""Combined contrastive + cross-entropy loss on 8 Trainium2 NeuronCores.

Math (matches the jax reference):
  d2[i,j]   = ||z_i||^2 + ||z_j||^2 - 2 z_i.z_j + 2*eps*(s_i - s_j) + D*eps^2
  dist      = sqrt(max(d2, 0))           (floor 1e-12 only matters on the
                                          diagonal, handled analytically)
  pair_loss = (1-same)*d2 + same*relu(1 - dist)^2
  contrastive = sum_{i<j} pair_loss / (N(N-1)/2)
  supervised  = mean CE(preds, labels)

Sharding: data-parallel over rows. Core c computes the 512 x 4096 block-row
of the pair matrix (its local latents against all latents) plus CE over its
512 rows of preds, and returns per-row-tile partial sums. The host sums the
per-core partials in float64 and applies the closed-form diagonal
correction (sum over ALL (i,j) minus diagonal, halved == sum i<j).

Device structure (per core):
  * gram in bf16 (latents quantized host-side; fp32 matmul is half-rate on
    the PE): PSUM P = (-2 z_loc).z_all + c_n, with the column constant
    c_n = ||z_n||^2 - 2 eps s_n + D eps^2 folded in as a K=1 bf16 ones-row
    matmul. The row constant b_m = ||z_m||^2 + 2 eps s_m (computed in f32
    from row-major latents) enters through the DVE clamp.
  * label-equality WITHOUT matmuls: a broadcast labels tile [128, N] against
    the per-partition local label via tensor_scalar(not_equal) on GpSimd ->
    1.0/0.0 mask, used directly as copy_predicated's nonzero predicate.
  * per [128,1024] chunk: DVE r = max(P + b, 0); ACT d = sqrt(r),
    h = relu(1-d); DVE copy_predicated(h, q!=0, d); ACT Square(h) with
    accum_out -> per-chunk row sums. 3 ACT + 2 DVE + 1 Pool passes.
  * CE batched [128, 4x1000]: Exp without max-shift (preds ~ N(0,1);
    log(sum exp p) == m + log(sum exp(p-m))), class-pick mask via
    is_equal against an iota tile, mul + row-reduce.
  * ACT table discipline: Relu/Square/Copy live in every table set; the two
    anchors (Exp/Ln for CE, Sqrt for pairwise) are kept in separate phases
    via tc.no_sync_barrier() so there are exactly two table loads.
"""

import numpy as np

_N, _D, _C = 4096, 256, 1000
_NCORES = 8
_ROWS = _N // _NCORES          # 512 rows per core
_RB = _ROWS // 128             # 4 row tiles per core
_CHUNK = 1024
_NG = _N // _CHUNK             # 4 column chunks
_EPS = 1e-6
_MARGIN = 1.0

_PROGS = {}  # compiled Bass programs keyed by reps


def _np_bf16():
    from concourse import mybir
    return mybir.dt.np(mybir.dt.bfloat16)


def _build_program(reps=1):
    """Build the SPMD program. reps>1 repeats the whole body (including
    input DMA) for wall-clock timing amplification in test.py."""
    import concourse.bacc as bacc
    import concourse.tile as tile
    from concourse import mybir

    f32 = mybir.dt.float32
    bf16 = mybir.dt.bfloat16
    i32 = mybir.dt.int32
    i16 = mybir.dt.int16
    i8 = mybir.dt.int8
    AF = mybir.ActivationFunctionType
    ALU = mybir.AluOpType
    AX = mybir.AxisListType

    nc = bacc.Bacc(
        "TRN2",
        target_bir_lowering=False,
        debug=False,
        enable_asserts=True,
        num_devices=_NCORES,
    )

    zT = nc.dram_tensor("zT", [_D, _N], bf16, kind="ExternalInput").ap()
    zlocT = nc.dram_tensor("zlocT", [_D, _ROWS], bf16, kind="ExternalInput").ap()
    zloc = nc.dram_tensor("zloc", [_ROWS, _D], f32, kind="ExternalInput").ap()
    lab_bc = nc.dram_tensor("lab_bc", [128, _N], i16, kind="ExternalInput").ap()
    lab_col = nc.dram_tensor("lab_col", [128, _RB], f32, kind="ExternalInput").ap()
    iota_bc = nc.dram_tensor("iota_bc", [128, _C], i16, kind="ExternalInput").ap()
    preds = nc.dram_tensor("preds", [_ROWS, _C], bf16, kind="ExternalInput").ap()
    out_pair = nc.dram_tensor("out_pair", [128, _RB * _NG], f32,
                              kind="ExternalOutput").ap()
    out_ce = nc.dram_tensor("out_ce", [128, _RB], f32, kind="ExternalOutput").ap()

    def emit(tc):
        with tc.tile_pool(name="const", bufs=1) as cpool, \
             tc.tile_pool(name="acc", bufs=1) as apool:
            # ---- resident inputs ------------------------------------------
            zTa = cpool.tile([128, _N], bf16)
            nc.sync.dma_start(zTa[:], zT[0:128, :])
            zTb = cpool.tile([128, _N], bf16)
            nc.sync.dma_start(zTb[:], zT[128:256, :])
            # both K-halves of the local (transposed, -2-scaled) latents in
            # one tile / one DMA: cols 0:512 = K rows 0:128, 512:1024 = rest
            zlT = cpool.tile([128, 2 * _ROWS], bf16)
            nc.sync.dma_start(
                zlT[:].rearrange("p (k r) -> p k r", k=2),
                zlocT.rearrange("(k p) r -> p k r", p=128))
            zloc_sb = cpool.tile([128, _RB * _D], f32)
            nc.scalar.dma_start(
                zloc_sb[:].rearrange("p (r d) -> p r d", r=_RB),
                zloc.rearrange("(r p) d -> p r d", p=128))
            lab_col_sb = cpool.tile([128, _RB], f32)
            nc.gpsimd.dma_start(lab_col_sb[:], lab_col[:])
            iota_bc_sb = cpool.tile([128, _C], i16)
            nc.gpsimd.dma_start(iota_bc_sb[:], iota_bc[:])
            lab_bc_sb = cpool.tile([128, _N], i16)
            nc.sync.dma_start(lab_bc_sb[:], lab_bc[:])
            ones_row = cpool.tile([1, 128], bf16)
            nc.vector.memset(ones_row[:], 1.0)
            ones_col = cpool.tile([128, 1], bf16)
            nc.vector.memset(ones_col[:], 1.0)
            c_bf = cpool.tile([1, _N], bf16)     # sq_n - 2 eps s_n + D eps^2
            b_sb = cpool.tile([128, _RB], f32)   # sq_m + 2 eps s_m
            pair_acc = apool.tile([128, _RB * _NG], f32)
            ce_acc = apool.tile([128, _RB], f32)

            # ---- CE + preamble phase (ACT anchors: Exp, Ln; Square/Copy
            # are in every ACT table set so they never force a set switch;
            # data deps keep the pairwise Sqrt stream after this phase) -----
            with tc.tile_pool(name="pre", bufs=2) as pre, \
                 tc.tile_pool(name="prepsum", bufs=2, space="PSUM") as prepsum:
                # ---- CE, batched as [128, RB*C] ---------------------------
                p_all = pre.tile([128, _RB * _C], bf16, tag="p_all", bufs=1)
                p_all3 = p_all[:].rearrange("p (r c) -> p r c", r=_RB)
                preds3 = preds.rearrange("(r p) c -> p r c", p=128)
                nc.sync.dma_start(p_all3[:, 0:2, :], preds3[:, 0:2, :])
                nc.sync.dma_start(p_all3[:, 2:4, :], preds3[:, 2:4, :])
                e_all = pre.tile([128, _RB * _C], f32, tag="e_all", bufs=1)
                nc.scalar.activation(e_all[:], p_all[:], AF.Exp)
                se4 = pre.tile([128, _RB], f32, tag="se4")
                nc.vector.tensor_reduce(
                    se4[:], e_all[:].rearrange("p (r c) -> p r c", r=_RB),
                    axis=AX.X, op=ALU.add)
                l4 = pre.tile([128, _RB], f32, tag="l4")
                nc.scalar.activation(l4[:], se4[:], AF.Ln)
                cm_all = pre.tile([128, _RB * _C], bf16, tag="cm_all", bufs=1)
                for rb in range(_RB):
                    nc.gpsimd.tensor_scalar(
                        cm_all[:, rb * _C:(rb + 1) * _C], iota_bc_sb[:],
                        lab_col_sb[:, rb:rb + 1], None, op0=ALU.is_equal)
                pm_all = pre.tile([128, _RB * _C], f32, tag="pm_all", bufs=1)
                nc.vector.tensor_mul(pm_all[:], p_all[:], cm_all[:])
                plab4 = pre.tile([128, _RB], f32, tag="plab4")
                nc.vector.tensor_reduce(
                    plab4[:], pm_all[:].rearrange("p (r c) -> p r c", r=_RB),
                    axis=AX.X, op=ALU.add)
                # supervised per row = log(sum exp p) - p[label]
                nc.vector.tensor_sub(ce_acc[:], l4[:], plab4[:])

                # ---- column stats: one PSUM group accumulates
                # sq_n - 2 eps s_n via a (-2 eps) ones-column ---------------
                ones_eps = pre.tile([128, 1], bf16, tag="ones_eps", bufs=1)
                nc.vector.memset(ones_eps[:], -2.0 * _EPS)
                for ct in range(_N // 512):
                    cs = slice(ct * 512, (ct + 1) * 512)
                    z2c = pre.tile([128, 512], bf16, tag="z2c")
                    nc.scalar.activation(z2c[:], zTa[:, cs], AF.Square)
                    z2c2 = pre.tile([128, 512], bf16, tag="z2c2")
                    nc.scalar.activation(z2c2[:], zTb[:, cs], AF.Square)
                    ps_c = prepsum.tile([1, 512], f32, tag="ps_c")
                    nc.tensor.matmul(ps_c[:], ones_col[:, 0:1], z2c[:],
                                     start=True, stop=False)
                    nc.tensor.matmul(ps_c[:], ones_col[:, 0:1], z2c2[:],
                                     start=False, stop=False)
                    nc.tensor.matmul(ps_c[:], ones_eps[:, 0:1], zTa[:, cs],
                                     start=False, stop=False)
                    nc.tensor.matmul(ps_c[:], ones_eps[:, 0:1], zTb[:, cs],
                                     start=False, stop=True)
                    # c = (sq - 2 eps s) + D eps^2, rounded to bf16
                    nc.scalar.activation(c_bf[0:1, cs], ps_c[:], AF.Copy,
                                         bias=float(_D) * _EPS * _EPS,
                                         scale=1.0)
                # per-row bias b (f32 path)
                for rb in range(_RB):
                    ds = slice(rb * _D, (rb + 1) * _D)
                    z2 = pre.tile([128, _D], f32, tag="z2")
                    nc.scalar.activation(z2[:], zloc_sb[:, ds], AF.Square)
                    sq_r = pre.tile([128, 1], f32, tag="sq_r")
                    nc.vector.reduce_sum(sq_r[:], z2[:], axis=AX.X)
                    s_r = pre.tile([128, 1], f32, tag="s_r")
                    nc.vector.reduce_sum(s_r[:], zloc_sb[:, ds], axis=AX.X)
                    s_sc = pre.tile([128, 1], f32, tag="s_sc")
                    nc.vector.tensor_scalar_mul(s_sc[:], s_r[:], 2.0 * _EPS)
                    nc.vector.tensor_add(b_sb[:, rb:rb + 1], sq_r[:], s_sc[:])

            # ---- pairwise block-row in [128, CHUNK] chunks ----------------
            # stage-major per row-block so each engine streams same-stage ops
            # back-to-back instead of ping-ponging down the chunk chain
            with tc.tile_pool(name="work", bufs=_NG) as wpool, \
                 tc.tile_pool(name="ppsum", bufs=3, space="PSUM") as ppsum:
                for rb in range(_RB):
                    rs = slice(rb * 128, (rb + 1) * 128)
                    pps, qts, dts, hts = [], [], [], []
                    for g in range(_NG):
                        pp = ppsum.tile([128, _CHUNK], f32, tag="pp")
                        pps.append(pp)
                        for h in range(_CHUNK // 512):
                            sl = slice(h * 512, (h + 1) * 512)
                            ncs = slice(g * _CHUNK + h * 512,
                                        g * _CHUNK + (h + 1) * 512)
                            nc.tensor.matmul(pp[:, sl], zlT[:, rs], zTa[:, ncs],
                                             start=True, stop=False)
                            nc.tensor.matmul(pp[:, sl],
                                             zlT[:, _ROWS + rb * 128:
                                                 _ROWS + (rb + 1) * 128],
                                             zTb[:, ncs],
                                             start=False, stop=False)
                            nc.tensor.matmul(pp[:, sl], ones_row[0:1, :],
                                             c_bf[0:1, ncs],
                                             start=False, stop=True)
                    for g in range(_NG):
                        # labels-differ mask (1.0 / 0.0) on GpSimd
                        q_t = wpool.tile([128, _CHUNK], i8, tag="q")
                        qts.append(q_t)
                        nc.gpsimd.tensor_scalar(
                            q_t[:], lab_bc_sb[:, g * _CHUNK:(g + 1) * _CHUNK],
                            lab_col_sb[:, rb:rb + 1], None, op0=ALU.not_equal)
                    rts = []
                    for g in range(_NG):
                        # clamp to bf16 SBUF: r = max(P + b, 0) = clamped d2
                        # (also frees the PSUM slot as early as possible)
                        r_t = wpool.tile([128, _CHUNK], f32, tag="rr")
                        rts.append(r_t)
                        nc.vector.tensor_scalar(
                            r_t[:], pps[g][:], b_sb[:, rb:rb + 1], 0.0,
                            op0=ALU.add, op1=ALU.max)
                    for g in range(_NG):
                        d_t = wpool.tile([128, _CHUNK], bf16, tag="d")
                        dts.append(d_t)
                        nc.scalar.activation(d_t[:], rts[g][:], AF.Sqrt)
                    for g in range(_NG):
                        # m = min(d, margin) - margin; m^2 == relu(margin-d)^2
                        h_t = wpool.tile([128, _CHUNK], bf16, tag="h")
                        hts.append(h_t)
                        nc.gpsimd.tensor_scalar(
                            h_t[:], dts[g][:], _MARGIN, _MARGIN,
                            op0=ALU.min, op1=ALU.subtract)
                    for g in range(_NG):
                        # labels differ -> dist; same -> min-form hinge
                        nc.vector.copy_predicated(
                            hts[g][:], qts[g][:], dts[g][:])
                    for g in range(_NG):
                        idx = rb * _NG + g
                        nc.scalar.activation(hts[g][:], hts[g][:], AF.Square,
                                             accum_out=pair_acc[:, idx:idx + 1])

            nc.sync.dma_start(out_pair[:], pair_acc[:])
            nc.sync.dma_start(out_ce[:], ce_acc[:])

    with tile.TileContext(nc) as tc:
        for _rep in range(reps):
            emit(tc)

    nc.compile()
    return nc


def _get_program(reps=1):
    if reps not in _PROGS:
        _PROGS[reps] = _build_program(reps)
    return _PROGS[reps]


def _prep_in_maps(latents, labels, preds):
    bf = _np_bf16()
    lat = np.ascontiguousarray(np.asarray(latents, dtype=np.float32))
    lab = np.asarray(labels).astype(np.int64)
    prd = np.ascontiguousarray(np.asarray(preds, dtype=np.float32))
    assert lat.shape == (_N, _D) and prd.shape == (_N, _C) and lab.shape == (_N,)

    zT_full = np.ascontiguousarray(lat.T)                      # [D, N] f32
    zT_bf = zT_full.astype(bf)
    labi = lab.astype(np.int16)                                # < 1000, exact
    lab_bc = np.ascontiguousarray(np.broadcast_to(labi[None, :], (128, _N)))
    iota_bc = np.ascontiguousarray(
        np.broadcast_to(np.arange(_C, dtype=np.int16)[None, :], (128, _C)))

    in_maps = []
    for c in range(_NCORES):
        sl = slice(c * _ROWS, (c + 1) * _ROWS)
        # lab_col[p, rb] = label of local row rb*128+p
        lab_col = np.ascontiguousarray(
            labi[sl].reshape(_RB, 128).T.astype(np.float32))
        in_maps.append({
            "zT": zT_bf,
            # pre-scale by -2 so the gram matmuls produce -2*G directly
            "zlocT": np.ascontiguousarray(-2.0 * zT_full[:, sl]).astype(bf),
            "zloc": np.ascontiguousarray(lat[sl]),
            "lab_bc": lab_bc,
            "lab_col": lab_col,
            "iota_bc": iota_bc,
            "preds": np.ascontiguousarray(prd[sl]).astype(bf),
        })
    return in_maps


def kernel(latents, labels, preds):
    from concourse.bass_utils import run_bass_kernel_spmd

    in_maps = _prep_in_maps(latents, labels, preds)
    nc = _get_program()
    res = run_bass_kernel_spmd(nc, in_maps, core_ids=list(range(_NCORES)))

    pair_sum = 0.0
    ce_sum = 0.0
    for r in res.results:
        pair_sum += float(r["out_pair"].astype(np.float64).sum())
        ce_sum += float(r["out_ce"].astype(np.float64).sum())

    # diagonal of the full-matrix sum: d2_ii = max(D*eps^2, 1e-12), same label
    d2ii = max(_D * _EPS * _EPS, 1e-12)
    hii = max(_MARGIN - np.sqrt(d2ii), 0.0)
    diag = _N * hii * hii
    contrastive = (pair_sum - diag) / (_N * (_N - 1.0))
    supervised = ce_sum / _N
    total = contrastive + supervised
    return (np.float32(total), np.float32(contrastive), np.float32(supervised))
